# revision 26
# baseline (speedup 1.0000x reference)
"""nn_CNNTransformer Bass/Tile kernel for 8 trn2 NeuronCores.

Sharding: core c -> (batch b = c//4, head-group hg = c%4, heads {2hg, 2hg+1}).
Each core: QK convs / attention / v-conv / unify partial for its 2 heads;
unify partials AllReduce'd (bf16) over the 4-core batch group; LN + MLP
replicated within the group. Final full-image output convs computed fully
per core (cheap); host takes core 0 / core 4 results.

Attention avoids materializing V: o = Conv_v(xmix) where
xmix[:, j] = sum_q wts[j, q] * x[:, q] (conv is linear in its input).

Device layouts (bf16 unless noted):
  t_pad SBUF [128=(ci, tile_par), 128 tp, 18, 18]   zero-padded tiles
  Q/K   DRAM [128=(h, ci), 2 par, 128 tp, 256 px]
  xT    DRAM [128 tp, 2 par, 64 ci, 256 px]         (PE transpose of t)
  wtsT  SBUF [128 tp_q, 2 par_q, 2 h, 256 j]        j order = (jb, par_j, tp_j)
  xmix  SBUF [128=(ci, h), 256 t, 18, 18]           padded
  u     DRAM [2 par, 128 tp, 64 ci, 256 px]         unify partials (AllReduce)
"""
import numpy as np
import ml_dtypes

import concourse.bacc as bacc
import concourse.mybir as mybir
import concourse.tile as tile
import concourse.bass as bass
from concourse import bass_utils

F32 = mybir.dt.float32
BF = mybir.dt.bfloat16
AX = mybir.AxisListType
AF = mybir.ActivationFunctionType

B, NT, TH, TW, HID, HEADS, L = 2, 16, 16, 16, 64, 8, 2
T, PX, TP = NT * NT, TH * TW, NT * NT // 2
PD = 18
PADT = PD * PD
N_CORES = 8
EPS = 1e-5
TAPS = [(dy, dx) for dy in range(3) for dx in range(3)]
BF_NP = ml_dtypes.bfloat16


def _bc(ap, extra):
    """Append a broadcast (step-0) dim of size `extra` to an AP."""
    return bass.AP(tensor=ap.tensor, offset=ap.offset, ap=list(ap.ap) + [[0, extra]])


def _bc_mid(ap, n):
    """Insert a broadcast dim of size n after the partition dim."""
    a = list(ap.ap)
    return bass.AP(tensor=ap.tensor, offset=ap.offset, ap=[a[0], [0, n]] + a[1:])


def _redim(ap, dims):
    """Replace the free dims of `ap` (keeping partition dim + offset)."""
    return bass.AP(tensor=ap.tensor, offset=ap.offset, ap=[ap.ap[0]] + dims)


def build(n_cores=N_CORES, n_layers=L, debug=(), stages=8, s8part=4):
    nc = bacc.Bacc()

    def din(name, shape, dt=BF):
        return nc.dram_tensor(name, shape, dt, kind="ExternalInput")

    x_tp = din("x_tp", [3, 2, TP, PX])
    pos_p = din("pos_p", [128, TP], F32)
    eye_i = din("eye_i", [128, 128], F32)
    mask_i = din("mask_i", [128, 2], F32)
    wsem_i = din("wsem_i", [3, HID])
    bsem_i = din("bsem_i", [128, 1], F32)
    wq_i = din("wq_i", [L, 128, 9, 128])
    wk_i = din("wk_i", [L, 128, 9, 128])
    wv_i = din("wv_i", [L, 128, 9, 64])
    wu_i = din("wu_i", [L, 128, 9, 64])
    w1_i = din("w1_i", [L, 128, 256])
    b1_i = din("b1_i", [L, 128, 2], F32)
    w2_i = din("w2_i", [L, 128, 2, 64])
    b2_i = din("b2_i", [L, 128, 1], F32)
    ln_i = din("ln_i", [L, 128, 2, 2, PX], F32)
    wo1_i = din("wo1_i", [128, 9, 64])
    bo1_i = din("bo1_i", [128, 1], F32)
    wo2_i = din("wo2_i", [128, 3])
    bo2_i = din("bo2_i", [128, 1], F32)

    img_o = nc.dram_tensor("img", [3, 256, 256], F32, kind="ExternalOutput")

    if n_cores >= 4:
        groups = [list(range(g * 4, g * 4 + 4)) for g in range(n_cores // 4)]
    else:
        groups = [list(range(n_cores))]

    import contextlib
    with tile.TileContext(nc) as tc, contextlib.ExitStack() as stack:
        wp = stack.enter_context(tc.tile_pool(name="wp", bufs=1))
        dramp = stack.enter_context(tc.tile_pool(name="dramp", bufs=1, space="DRAM"))

        def load(dram_ap, shape, name, dt=BF):
            t = wp.tile(shape, dt, name=name)
            nc.sync.dma_start(t[:], dram_ap)
            return t

        wsem = load(wsem_i[:], [3, HID], "wsem")
        wq = [load(wq_i[l], [128, 9, 128], f"wq{l}") for l in range(n_layers)]
        wk = [load(wk_i[l], [128, 9, 128], f"wk{l}") for l in range(n_layers)]
        wv = [load(wv_i[l], [128, 9, 64], f"wv{l}") for l in range(n_layers)]
        wu = [load(wu_i[l], [128, 9, 64], f"wu{l}") for l in range(n_layers)]
        w1 = [load(w1_i[l], [128, 256], f"w1{l}") for l in range(n_layers)]
        w2 = [load(w2_i[l], [128, 2, 64], f"w2{l}") for l in range(n_layers)]
        b1v = [load(b1_i[l], [128, 2], f"b1v{l}", F32) for l in range(n_layers)]
        b2v = [load(b2_i[l], [128, 1], f"b2v{l}", F32) for l in range(n_layers)]
        lnw = [load(ln_i[l], [128, 2, 2, PX], f"lnw{l}", F32)
               for l in range(n_layers)]
        wo1 = load(wo1_i[:], [128, 9, 64], "wo1")
        wo2 = load(wo2_i[:], [128, 3], "wo2")
        eye_f = load(eye_i[:], [128, 128], "eye_ff", F32)
        eye = wp.tile([128, 128], BF, name="eye")
        nc.vector.tensor_copy(eye[:], eye_f[:])
        mask = load(mask_i[:], [128, 2], "mask", F32)
        bsem = load(bsem_i[:], [128, 1], "bsem", F32)
        bo1 = load(bo1_i[:], [128, 1], "bo1", F32)
        bo2 = load(bo2_i[:], [128, 1], "bo2", F32)
        pos_s = load(pos_p[:], [128, TP], "pos_s", F32)

        t_dram = [dramp.tile([128, TP, PD, PD], BF, name=f"t_dram{l}")
                  for l in range(n_layers + 1)]
        q_dram = dramp.tile([128, 2, TP, PX], BF, name="q_dram")
        k_dram = dramp.tile([128, 2, TP, PX], BF, name="k_dram")
        xT_dram = dramp.tile([128, 2, HID, PX], BF, name="xT_dram")
        u_dram = dramp.tile([2, TP, HID, PX], BF, name="u_dram")
        u_ar = dramp.tile([2, TP, HID, PX], BF, name="u_ar")

        MM = nc.tensor.matmul

        # =================== S1: sem conv + pos ===================
        def stage1(t_pad):
            with tc.tile_pool(name="s1x", bufs=1) as xp, \
                 tc.tile_pool(name="s1", bufs=2) as sp, \
                 tc.tile_pool(name="s1p", bufs=4, space="PSUM") as pp:
                for par in range(2):
                    rows = slice(64 * par, 64 * par + 64)
                    xs = xp.tile([3, TP, PX], BF, name="xs", tag="xs")
                    nc.sync.dma_start(xs[:], x_tp[:, par, :, :])
                    for r in range(TP // 2):
                        ps = pp.tile([64, 2, PX], F32, name="s1ps")
                        MM(ps[:], wsem[:], xs[0:3, 2 * r:2 * r + 2, :],
                           start=True, stop=True)
                        st = sp.tile([128, 2, PX], F32, name="s1st")
                        nc.scalar.activation(st[rows], ps[:], AF.Relu,
                                             bias=bsem[rows])
                        dst = t_pad[rows, 2 * r:2 * r + 2, 1:17, 1:17]
                        nc.vector.tensor_add(
                            dst, st[rows].rearrange("p a (y x) -> p a y x", y=16),
                            _bc(pos_s[rows, 2 * r:2 * r + 2], PX).rearrange(
                                "p a (y x) -> p a y x", y=16))
                nc.sync.dma_start(t_dram[0][:], t_pad[:])

        # =================== S2: Q/K convs ===================
        def conv_qk(l, t_pad, w_l, out_dram):
            with tc.tile_pool(name="cqk", bufs=4) as sp, \
                 tc.tile_pool(name="cqkp", bufs=2, space="PSUM") as pp:
                for r in range(TP // 2):
                    psE = pp.tile([128, 2, PX], F32, name="qkpsE")
                    psO = pp.tile([128, 2, PX], F32, name="qkpsO")
                    for ti, (dy, dx) in enumerate(TAPS):
                        st_, sp_ = (ti == 0), (ti == 8)
                        MM(psE[:], w_l[0:64, ti, :],
                           t_pad[0:64, 2 * r:2 * r + 2, dy:dy + 16, dx:dx + 16],
                           start=st_, stop=sp_)
                        MM(psO[:], w_l[64:128, ti, :],
                           t_pad[64:128, 2 * r:2 * r + 2, dy:dy + 16, dx:dx + 16],
                           start=st_, stop=sp_)
                    stE = sp.tile([128, 2, PX], BF, name="qkstE")
                    stO = sp.tile([128, 2, PX], BF, name="qkstO")
                    nc.vector.tensor_copy(stE[:], psE[:])
                    nc.scalar.copy(stO[:], psO[:])
                    nc.sync.dma_start(out_dram[:, 0, 2 * r:2 * r + 2, :], stE[:])
                    nc.sync.dma_start(out_dram[:, 1, 2 * r:2 * r + 2, :], stO[:])

        # =================== S3: xT via PE transpose ===================
        def stage3(l, t_pad):
            with tc.tile_pool(name="s3", bufs=3) as sp, \
                 tc.tile_pool(name="s3p", bufs=4, space="PSUM") as pp:
                for blk in range(8):
                    stg = sp.tile([128, 2, HID, 32], BF, name="xtstg")
                    for k in range(32):
                        px = blk * 32 + k
                        y, x = px // 16, px % 16
                        tps = pp.tile([128, 128], BF, name="xtps")
                        nc.tensor.transpose(tps[:], t_pad[:, :, 1 + y, 1 + x], eye[:])
                        src = tps[:].rearrange("q (a c) -> q a c", a=2)
                        if k % 2 == 0:
                            nc.vector.tensor_copy(stg[:, :, :, k], src)
                        else:
                            nc.scalar.copy(stg[:, :, :, k], src)
                    nc.sync.dma_start(xT_dram[:, :, :, blk * 32:blk * 32 + 32], stg[:])

        # ============== S4/S5: scores + softmax + wts transpose ============
        def stage45(l, wts_sb):
            with tc.tile_pool(name="s4k", bufs=1) as kp, \
                 tc.tile_pool(name="s4q", bufs=1) as qp, \
                 tc.tile_pool(name="s4", bufs=2) as sp, \
                 tc.tile_pool(name="s4p", bufs=1, space="PSUM") as pp, \
                 tc.tile_pool(name="s4p2", bufs=2, space="PSUM") as pp2:
                for jb in range(2):
                    pss = [pp.tile([128, T], F32, name=f"ps_s{h}") for h in range(2)]
                    for pxh in range(2):
                        k_sb = kp.tile([128, 2, TP, 128], BF, name="k_sb")
                        q_sb = qp.tile([128, 2, 64, 128], BF, name="q_sb")
                        for par in range(2):
                            nc.sync.dma_start(
                                k_sb[:, par], k_dram[:, par, :,
                                                     128 * pxh:128 * pxh + 128])
                            nc.sync.dma_start(
                                q_sb[:, par], q_dram[:, par, 64 * jb:64 * jb + 64,
                                                     128 * pxh:128 * pxh + 128])
                        for pxi in range(128):
                            px = 128 * pxh + pxi
                            for h in range(2):
                                rows = slice(64 * h, 64 * h + 64)
                                MM(pss[h][:], q_sb[rows, :, :, pxi],
                                   k_sb[rows, :, :, pxi],
                                   start=(px == 0), stop=(px == PX - 1),
                                   skip_group_check=True)
                    for h in range(2):
                        mx = sp.tile([128, 1], F32, name="mx")
                        nc.vector.reduce_max(mx[:], pss[h][:], axis=AX.X)
                        nb = sp.tile([128, 1], F32, name="nb")
                        nc.vector.tensor_scalar_mul(nb[:], mx[:], -1.0 / 16.0)
                        e = sp.tile([128, T], F32, name="e")
                        nc.scalar.activation(e[:], pss[h][:], AF.Exp,
                                             bias=nb[:], scale=1.0 / 16.0)
                        s = sp.tile([128, 1], F32, name="s")
                        nc.vector.reduce_sum(s[:], e[:], axis=AX.X)
                        rv = sp.tile([128, 1], F32, name="rv")
                        nc.vector.reciprocal(rv[:], s[:])
                        w_row = sp.tile([128, T], BF, name="w_row")
                        nc.vector.tensor_scalar_mul(w_row[:], e[:], rv[:])
                        for par in range(2):
                            tps = pp2.tile([128, 128], BF, name="wtps")
                            nc.tensor.transpose(
                                tps[:], w_row[:, 128 * par:128 * par + 128], eye[:])
                            nc.scalar.copy(
                                wts_sb[:, par, h, 128 * jb:128 * jb + 128], tps[:])

        # =================== S6: xmix = x @ wtsT ===================
        def stage6(l, wts_sb, xmix_pad):
            with tc.tile_pool(name="s6", bufs=2) as sp, \
                 tc.tile_pool(name="s6p", bufs=4, space="PSUM") as pp:
                nc.gpsimd.memset(xmix_pad[:, :, 0, :], 0.0)
                nc.gpsimd.memset(xmix_pad[:, :, 17, :], 0.0)
                nc.gpsimd.memset(xmix_pad[:, :, 1:17, 0], 0.0)
                nc.gpsimd.memset(xmix_pad[:, :, 1:17, 17], 0.0)
                for blk in range(8):
                    xt = sp.tile([128, 2, HID, 32], BF, name="xtblk")
                    nc.sync.dma_start(xt[:], xT_dram[:, :, :, blk * 32:blk * 32 + 32])
                    for k in range(32):
                        px = blk * 32 + k
                        y, x = px // 16, px % 16
                        ps = pp.tile([128, T], F32, name="psx")
                        for par in range(2):
                            for h in range(2):
                                MM(ps[64 * h:64 * h + 64],
                                   xt[:, par, :, k], wts_sb[:, par, h, :],
                                   start=(par == 0), stop=(par == 1),
                                   skip_group_check=True)
                        # psum col c = (jb, par_j, tp_j) -> tile t = 128jb+2tpj+par_j
                        for jbv in range(2):
                            base = xmix_pad[:, 128 * jbv, 1 + y, 1 + x]
                            dst = _redim(base, [[PADT, 2], [2 * PADT, 64]])
                            sl_ = ps[:, 128 * jbv:128 * jbv + 128]
                            if (k + jbv) % 2 == 0:
                                nc.vector.tensor_copy(dst, sl_)
                            else:
                                nc.scalar.copy(dst, sl_)

        # =================== S7a: vconv + unify + AllReduce ===============
        def stage7a(l, xmix_pad):
            with tc.tile_pool(name="s7a", bufs=3) as sp, \
                 tc.tile_pool(name="s7ap", bufs=3, space="PSUM") as pp, \
                 tc.tile_pool(name="s7o", bufs=1) as op:
                opads = [op.tile([128, 8, PD, PD], BF, name=f"opad{i}")
                         for i in range(2)]
                for o in opads:
                    nc.gpsimd.memset(o[:], 0.0)
                for b in range(T // 8):
                    opad = opads[b % 2]
                    for rr in range(4):
                        t0 = 8 * b + 2 * rr
                        ps = pp.tile([128, 2, PX], F32, name="vps")
                        for ti, (dy, dx) in enumerate(TAPS):
                            st_, sp_ = (ti == 0), (ti == 8)
                            MM(ps[0:64], wv[l][0:64, ti, :],
                               xmix_pad[0:64, t0:t0 + 2, dy:dy + 16, dx:dx + 16],
                               start=st_, stop=sp_, skip_group_check=True)
                            MM(ps[64:128], wv[l][64:128, ti, :],
                               xmix_pad[64:128, t0:t0 + 2, dy:dy + 16, dx:dx + 16],
                               start=st_, stop=sp_, skip_group_check=True)
                        dst = opad[:, 2 * rr:2 * rr + 2, 1:17, 1:17]
                        src4 = ps[:].rearrange("p a (y x) -> p a y x", y=16)
                        if rr % 2 == 0:
                            nc.vector.tensor_copy(dst, src4)
                        else:
                            nc.scalar.copy(dst, src4)
                    for rr in range(2):
                        ps = pp.tile([128, 2, PX], F32, name="ups")
                        s0 = 4 * rr
                        for ti, (dy, dx) in enumerate(TAPS):
                            st_, sp_ = (ti == 0), (ti == 8)
                            MM(ps[0:64], wu[l][:, ti, :],
                               opad[:, s0:s0 + 2, dy:dy + 16, dx:dx + 16],
                               start=st_, stop=sp_, skip_group_check=True)
                            MM(ps[64:128], wu[l][:, ti, :],
                               opad[:, s0 + 2:s0 + 4, dy:dy + 16, dx:dx + 16],
                               start=st_, stop=sp_, skip_group_check=True)
                        st = sp.tile([128, 2, PX], BF, name="ust")
                        if rr == 0:
                            nc.vector.tensor_copy(st[:], ps[:])
                        else:
                            nc.scalar.copy(st[:], ps[:])
                        for k in range(4):
                            t = 8 * b + 4 * rr + k
                            src = st[64 * (k // 2):64 * (k // 2) + 64, k % 2, :]
                            nc.sync.dma_start(u_dram[t % 2, t // 2, :, :], src)
            nc.gpsimd.collective_compute(
                "AllReduce", mybir.AluOpType.add, replica_groups=groups,
                ins=[u_dram.opt()], outs=[u_ar.opt()])

        # ============ S7b: residual + LN1 + MLP + LN2 ============
        def ln_stats(red1, red2, pp, sp, pref):
            ps1 = pp.tile([128, 2], F32, name=pref + "st1", tag="lnst1")
            ps2 = pp.tile([128, 2], F32, name=pref + "st2", tag="lnst2")
            MM(ps1[:], red1[:], mask[:], start=True, stop=True)
            MM(ps2[:], red2[:], mask[:], start=True, stop=True)
            mu = sp.tile([128, 2], F32, name=pref + "mu")
            nc.vector.tensor_scalar_mul(mu[:], ps1[:], 1.0 / (HID * PX))
            ex2 = sp.tile([128, 2], F32, name=pref + "ex2")
            nc.vector.tensor_scalar_mul(ex2[:], ps2[:], 1.0 / (HID * PX))
            var = sp.tile([128, 2], F32, name=pref + "var")
            nc.vector.tensor_mul(var[:], mu[:], mu[:])
            nc.vector.tensor_sub(var[:], ex2[:], var[:])
            nc.vector.tensor_scalar_add(var[:], var[:], EPS)
            sd = sp.tile([128, 2], F32, name=pref + "sd")
            nc.scalar.sqrt(sd[:], var[:])
            rsmu = sp.tile([128, 4], F32, name=pref + "rsmu")
            nc.vector.reciprocal(rsmu[:, 0:2], sd[:])
            nc.vector.tensor_mul(rsmu[:, 2:4], mu[:], rsmu[:, 0:2])
            tps = pp.tile([4, 128], F32, name=pref + "tps", tag="lntps")
            nc.tensor.transpose(tps[:], rsmu[:], eye_f[:])
            rt = sp.tile([4, 128], F32, name=pref + "rt")
            nc.vector.tensor_copy(rt[:], tps[:])
            rt_d = dramp.tile([4, 128], F32, name=pref + "rt_d", tag="lnrt_d")
            nc.sync.dma_start(rt_d[:], rt[:])
            A_bc = sp.tile([128, TP], F32, name=pref + "A")
            muA = sp.tile([128, TP], F32, name=pref + "muA")
            for par in range(2):
                d = slice(64 * par, 64 * par + 64)
                nc.sync.dma_start(A_bc[d, :],
                                  rt_d[par:par + 1, :].partition_broadcast(64))
                nc.sync.dma_start(muA[d, :],
                                  rt_d[2 + par:3 + par, :].partition_broadcast(64))
            return A_bc, muA

        def stage7b(l, t_pad_next):
            NB = 2

            def normalize(dst, src_ap, A_bc, muA, li, sp, b0, dst4d=False):
                sl = slice(b0, b0 + NB)
                scr = sp.tile([128, NB, PX], F32, name="nscr")
                nc.vector.tensor_mul(scr[:], src_ap, _bc(A_bc[:, sl], PX))
                nc.vector.tensor_sub(scr[:], scr[:], _bc(muA[:, sl], PX))
                nc.vector.tensor_mul(scr[:], scr[:], _bc_mid(lnw[l][:, li, 0, :], NB))
                bia = _bc_mid(lnw[l][:, li, 1, :], NB)
                src = scr[:]
                if dst4d:
                    src = src.rearrange("p a (y x) -> p a y x", y=16)
                    bia = bia.rearrange("p a (y x) -> p a y x", y=16)
                nc.vector.tensor_add(dst, src, bia)

            with tc.tile_pool(name="s7b", bufs=2) as sp, \
                 tc.tile_pool(name="s7bp", bufs=2, space="PSUM") as pp, \
                 tc.tile_pool(name="s7bps", bufs=1, space="PSUM") as pps, \
                 tc.tile_pool(name="s7r", bufs=1) as rp:
                I1 = rp.tile([128, TP, PX], BF, name="I1")
                red1 = rp.tile([128, TP], F32, name="red1")
                red2 = rp.tile([128, TP], F32, name="red2")
                for b0 in range(0, TP, NB):
                    sl = slice(b0, b0 + NB)
                    ub = sp.tile([128, NB, PX], BF, name="ub")
                    for par in range(2):
                        src = u_ar[par, sl, :, :].rearrange("t c p -> c t p")
                        nc.sync.dma_start(ub[64 * par:64 * par + 64, :, :], src)
                    tr = sp.tile([128, NB, 16, 16], BF, name="tr")
                    for kk in range(NB):
                        nc.sync.dma_start(tr[:, kk], t_dram[l][:, b0 + kk, 1:17, 1:17])
                    nc.vector.tensor_add(I1[:, sl, :], ub[:],
                                         tr[:].rearrange("p a y x -> p a (y x)"))
                    sq = sp.tile([128, NB, PX], BF, name="sq")
                    nc.vector.tensor_mul(sq[:], I1[:, sl, :], I1[:, sl, :])
                    nc.vector.reduce_sum(red1[:, sl], I1[:, sl, :], axis=AX.X)
                    nc.vector.reduce_sum(red2[:, sl], sq[:], axis=AX.X)
                A1, muA1 = ln_stats(red1, red2, pps, sp, "l1")
                for b0 in range(0, TP, NB):
                    sl = slice(b0, b0 + NB)
                    i1n = sp.tile([128, NB, PX], BF, name="i1n")
                    normalize(i1n[:], I1[:, sl, :], A1, muA1, 0, sp, b0)
                    m_blk = [sp.tile([128, 2 * NB, PX], BF, name=f"mblk{ch}")
                             for ch in range(2)]
                    for par in range(2):
                        rows = slice(64 * par, 64 * par + 64)
                        for ch in range(2):
                            psm = pp.tile([128, NB, PX], F32, name="psm")
                            MM(psm[:], w1[l][rows, 128 * ch:128 * ch + 128],
                               i1n[rows, :, :], start=True, stop=True)
                            nc.scalar.activation(
                                m_blk[ch][:, NB * par:NB * par + NB, :], psm[:],
                                AF.Relu, bias=b1v[l][:, ch:ch + 1])
                    psw = pp.tile([128, NB, PX], F32, name="psw")
                    for ch in range(2):
                        for par in range(2):
                            MM(psw[64 * par:64 * par + 64], w2[l][:, ch, :],
                               m_blk[ch][:, NB * par:NB * par + NB, :],
                               start=(ch == 0), stop=(ch == 1),
                               skip_group_check=True)
                    mo = sp.tile([128, NB, PX], F32, name="mo")
                    nc.vector.tensor_scalar_add(mo[:], psw[:], b2v[l][:])
                    nc.vector.tensor_add(I1[:, sl, :], mo[:], i1n[:])
                    sq = sp.tile([128, NB, PX], BF, name="sq2")
                    nc.vector.tensor_mul(sq[:], I1[:, sl, :], I1[:, sl, :])
                    nc.vector.reduce_sum(red1[:, sl], I1[:, sl, :], axis=AX.X)
                    nc.vector.reduce_sum(red2[:, sl], sq[:], axis=AX.X)
                A2, muA2 = ln_stats(red1, red2, pps, sp, "l2")
                for b0 in range(0, TP, NB):
                    sl = slice(b0, b0 + NB)
                    dst = t_pad_next[:, sl, 1:17, 1:17]
                    normalize(dst, I1[:, sl, :], A2, muA2, 1, sp, b0,
                              dst4d=True)
                nc.sync.dma_start(t_dram[l + 1][:], t_pad_next[:])

        # =================== S8: full-image output convs ==================
        def stage8(s8part=4):
            tl = t_dram[n_layers]
            with tc.tile_pool(name="s8i", bufs=1) as ip, \
                 tc.tile_pool(name="s8", bufs=4) as sp, \
                 tc.tile_pool(name="s8p", bufs=2, space="PSUM") as pp:
                pimg = ip.tile([128, 130, 258], BF, name="pimg")
                img1 = ip.tile([128, 128, 256], BF, name="img1")
                nc.gpsimd.memset(pimg[:], 0.0)
                # row g (0..255) at partition-half g%2, slot (g+2)//2, col 1+x
                for ty in range(16):
                    for txp in range(2):
                        for yp in range(2):
                            for tk in range(8):
                                # src: tile tp=8ty+tk (tx par txp), y=yp::2, x
                                sbase = tl[64 * txp, 8 * ty + tk, 1 + yp, 1]
                                src = bass.AP(
                                    tensor=sbase.tensor, offset=sbase.offset,
                                    ap=[[TP * PADT, 64], [2 * PD, 8], [1, 16]])
                                dbase = pimg[64 * yp:64 * yp + 64,
                                             8 * ty + 1, 1 + 16 * txp + 32 * tk]
                                dst = _redim(dbase, [[258, 8], [1, 16]])
                                nc.sync.dma_start(dst, src)
                if s8part < 1:
                    return
                nrp = 128 if s8part >= 3 else 1
                for rp in range(nrp):
                    psA = pp.tile([128, PX], F32, name="s8psA")
                    psB = pp.tile([128, PX], F32, name="s8psB")
                    fA = [True, True]
                    fB = [True, True]
                    for ti, (dy, dx) in enumerate(TAPS):
                        for op_ in range(2):
                            g = 2 * rp + op_
                            ge = g - 1 + dy
                            pg, slot = ge % 2, (ge + 2) // 2
                            ps_, first = (psA, fA) if pg == 0 else (psB, fB)
                            MM(ps_[64 * op_:64 * op_ + 64],
                               wo1[64 * pg:64 * pg + 64, ti, :],
                               pimg[64 * pg:64 * pg + 64, slot, dx:dx + 256],
                               start=first[op_], stop=(ti >= 6),
                               skip_group_check=True)
                            first[op_] = False
                    sA = sp.tile([128, PX], F32, name="s8sA")
                    nc.scalar.copy(sA[:], psA[:])
                    tmp = sp.tile([128, PX], F32, name="s8tmp")
                    nc.vector.tensor_add(tmp[:], sA[:], psB[:])
                    nc.scalar.activation(img1[:, rp, :], tmp[:], AF.Relu,
                                         bias=bo1[:])
                if s8part < 4:
                    return
                for rp in range(128):
                    ps = pp.tile([128, PX], F32, name="s8ps2")
                    MM(ps[0:3], wo2[0:64, :], img1[0:64, rp, :],
                       start=True, stop=True, skip_group_check=True)
                    MM(ps[32:35], wo2[64:128, :], img1[64:128, rp, :],
                       start=True, stop=True, skip_group_check=True)
                    stE = sp.tile([3, PX], F32, name="s8stE")
                    stO = sp.tile([3, PX], F32, name="s8stO")
                    nc.vector.tensor_scalar_add(stE[:], ps[0:3], bo2[0:3])
                    nc.vector.tensor_scalar_add(stO[:], ps[32:35], bo2[0:3])
                    nc.sync.dma_start(img_o[:, 2 * rp, :], stE[:])
                    nc.sync.dma_start(img_o[:, 2 * rp + 1, :], stO[:])

        # =================== driver ===================
        import contextlib as _ctl
        tp_stack = _ctl.ExitStack()
        tpp = tp_stack.enter_context(tc.tile_pool(name="tp0", bufs=1))
        t_pad_cur = tpp.tile([128, TP, PD, PD], BF, name="tpad0")
        nc.gpsimd.memset(t_pad_cur[:], 0.0)
        stage1(t_pad_cur)
        for l in range(n_layers if stages >= 2 else 0):
            conv_qk(l, t_pad_cur, wq[l], q_dram)
            conv_qk(l, t_pad_cur, wk[l], k_dram)
            if stages >= 3:
                stage3(l, t_pad_cur)
            if stages < 4:
                break
            tp_stack.close()
            with tc.tile_pool(name=f"wtsp{l}", bufs=1) as wtp:
                wts_sb = wtp.tile([128, 2, 2, T], BF, name="wts_sb")
                stage45(l, wts_sb)
                with tc.tile_pool(name=f"xmp{l}", bufs=1) as xmp:
                    xmix_pad = xmp.tile([128, T, PD, PD], BF, name="xmix_pad")
                    if stages >= 5:
                        stage6(l, wts_sb, xmix_pad)
                    if stages >= 6:
                        stage7a(l, xmix_pad)
            tp_stack = _ctl.ExitStack()
            tpp = tp_stack.enter_context(tc.tile_pool(name=f"tp{l + 1}", bufs=1))
            t_pad_next = tpp.tile([128, TP, PD, PD], BF, name=f"tpad{l + 1}")
            nc.gpsimd.memset(t_pad_next[:], 0.0)
            if stages >= 7:
                stage7b(l, t_pad_next)
            t_pad_cur = t_pad_next
        tp_stack.close()
        if stages >= 8:
            stage8(s8part)

        for name in debug:
            src = {"t0": t_dram[0], "t1": t_dram[1],
                   "tl": t_dram[n_layers],
                   "q": q_dram, "k": k_dram, "xT": xT_dram,
                   "u": u_dram, "uar": u_ar}[name]
            d = nc.dram_tensor("dbg_" + name, list(src.shape), BF,
                               kind="ExternalOutput")
            nc.sync.dma_start(d[:], src[:])

    if not nc.is_finalized():
        nc.finalize()
    return nc


# ======================= host side =======================

def _host_pack(inputs, core):
    b, hg = core // 4, core % 4
    f = lambda a: np.ascontiguousarray(np.asarray(a, np.float32))
    bf = lambda a: np.ascontiguousarray(a).astype(BF_NP)
    x = f(inputs["x"])
    pos = f(inputs["pos"])
    out = {}
    xt = x[b].reshape(3, NT, TH, NT, TW).transpose(0, 1, 3, 2, 4).reshape(3, T, PX)
    out["x_tp"] = bf(xt.reshape(3, TP, 2, PX).transpose(0, 2, 1, 3))
    posf = pos.reshape(T, HID)
    pp = np.zeros((128, TP), np.float32)
    for par in range(2):
        pp[64 * par:64 * par + 64, :] = posf[par::2, :].T
    out["pos_p"] = pp
    out["eye_i"] = np.eye(128, dtype=np.float32)
    mk = np.zeros((128, 2), np.float32)
    mk[0:64, 0] = 1.0
    mk[64:128, 1] = 1.0
    out["mask_i"] = mk
    out["wsem_i"] = bf(f(inputs["sem_w"])[:, :, 0, 0].T)
    bs = np.zeros((128, 1), np.float32)
    bs[0:64, 0] = f(inputs["sem_b"])
    bs[64:128, 0] = f(inputs["sem_b"])
    out["bsem_i"] = bs

    co0 = 128 * hg
    for nm, w in (("wq_i", f(inputs["qw"])), ("wk_i", f(inputs["kw"]))):
        p = np.zeros((L, 128, 9, 128), np.float32)
        sl = w[:, co0:co0 + 128]                                  # [L,128co,64,3,3]
        lhsT = sl.transpose(0, 2, 3, 4, 1).reshape(L, HID, 9, 128)
        p[:, 0:64] = lhsT
        p[:, 64:128] = lhsT
        out[nm] = bf(p)
    vw = f(inputs["vw"])
    p = np.zeros((L, 128, 9, 64), np.float32)
    for h in range(2):
        sl = vw[:, co0 + 64 * h:co0 + 64 * h + 64]                # [L,64,64,3,3]
        p[:, 64 * h:64 * h + 64] = sl.transpose(0, 2, 3, 4, 1).reshape(L, HID, 9, 64)
    out["wv_i"] = bf(p)
    uw = f(inputs["uw"])                                          # [L,64,512,3,3]
    p = uw[:, :, co0:co0 + 128].transpose(0, 2, 3, 4, 1).reshape(L, 128, 9, 64)
    out["wu_i"] = bf(p)
    w1 = f(inputs["mlp_w1"])[:, :, :, 0, 0]                       # [L,256,64]
    p = np.zeros((L, 128, 256), np.float32)
    p[:, 0:64] = w1.transpose(0, 2, 1)
    p[:, 64:128] = w1.transpose(0, 2, 1)
    out["w1_i"] = bf(p)
    b1 = f(inputs["mlp_b1"])                                      # [L,256]
    out["b1_i"] = np.ascontiguousarray(
        b1.reshape(L, 2, 128).transpose(0, 2, 1)).astype(np.float32)
    w2 = f(inputs["mlp_w2"])[:, :, :, 0, 0]                       # [L,64,256]
    out["w2_i"] = bf(w2.transpose(0, 2, 1).reshape(L, 2, 128, 64)
                     .transpose(0, 2, 1, 3))
    b2 = np.zeros((L, 128, 1), np.float32)
    b2[:, 0:64, 0] = f(inputs["mlp_b2"])
    b2[:, 64:128, 0] = f(inputs["mlp_b2"])
    out["b2_i"] = b2
    ln = np.zeros((L, 128, 2, 2, PX), np.float32)
    for li, (wn, bn) in enumerate((("ln1_w", "ln1_b"), ("ln2_w", "ln2_b"))):
        wv_ = f(inputs[wn]).reshape(L, HID, PX)
        bv_ = f(inputs[bn]).reshape(L, HID, PX)
        for dup in range(2):
            ln[:, 64 * dup:64 * dup + 64, li, 0] = wv_
            ln[:, 64 * dup:64 * dup + 64, li, 1] = bv_
    out["ln_i"] = ln
    wo1 = f(inputs["out_w1"])                                     # [64,64,3,3]
    p = np.zeros((128, 9, 64), np.float32)
    lhsT = wo1.transpose(1, 2, 3, 0).reshape(HID, 9, 64)
    p[0:64] = lhsT
    p[64:128] = lhsT
    out["wo1_i"] = bf(p)
    bo1 = np.zeros((128, 1), np.float32)
    bo1[0:64, 0] = f(inputs["out_b1"])
    bo1[64:128, 0] = f(inputs["out_b1"])
    out["bo1_i"] = bo1
    wo2 = f(inputs["out_w2"])[:, :, 0, 0]                         # [3,64]
    p = np.zeros((128, 3), np.float32)
    p[0:64] = wo2.T
    p[64:128] = wo2.T
    out["wo2_i"] = bf(p)
    bo2 = np.zeros((128, 1), np.float32)
    bo2[0:3, 0] = f(inputs["out_b2"])
    bo2[32:35, 0] = f(inputs["out_b2"])
    out["bo2_i"] = bo2
    return out


_NC_CACHE = {}


def _get_nc(n_cores=N_CORES, n_layers=L, debug=(), stages=8, s8part=4):
    key = (n_cores, n_layers, tuple(debug), stages, s8part)
    if key not in _NC_CACHE:
        _NC_CACHE[key] = build(n_cores, n_layers, debug, stages, s8part)
    return _NC_CACHE[key]


def kernel(**inputs):
    nc = _get_nc()
    in_maps = [_host_pack(inputs, c) for c in range(N_CORES)]
    r = bass_utils.run_bass_kernel_spmd(nc, in_maps, core_ids=list(range(N_CORES)))
    out = np.zeros((B, 3, 256, 256), np.float32)
    out[0] = r.results[0]["img"]
    out[1] = r.results[4]["img"]
    return out
